# revision 8
# baseline (speedup 1.0000x reference)
"""Trainium2 Bass kernel for nn_GCN (CNN encoder + 6 ARMA graph-conv layers).

Sharding: nodes (and their images) split across 8 NeuronCores, 625 each
(padded to 640).  Weights replicated.  Per ARMA layer the node features are
exchanged with a bf16 AllGather; the normalized adjacency is a dense bf16
matrix built on-device from a host-uploaded edge-multiplicity matrix and
device-computed deg^-1/2 scalings.
"""

import sys

for _p in ("/opt/trn_rl_repo", "/root/.axon_site/_ro/trn_rl_repo"):
    if _p not in sys.path:
        sys.path.append(_p)

import numpy as np
import ml_dtypes

import concourse.bacc as bacc
import concourse.bass as bass
import concourse.mybir as mybir
import concourse.tile as tile
from concourse import bass_utils

BF16 = mybir.dt.bfloat16
F32 = mybir.dt.float32
AF = mybir.ActivationFunctionType
ALU = mybir.AluOpType

N_NODES = 5000
N_CORES = 8
NLOC = 625            # real nodes per core
NPAD = 640            # padded nodes per core
NG = N_CORES * NPAD   # 5120 padded global nodes
BLK = 64              # images per CNN block
NBLK = NPAD // BLK    # 10
NCH = NG // 128       # 40 A-chunks

# CNN geometry
IN_PIX = 318          # 53*6
X1W = 13056           # (x 4)(y 51)(i 64)
X2W = 9472            # 147*64 (+64 pad), cols (x0*49+y0)*64+i
X2V = 9408
NT2 = X2W // 128      # 74 tiles after xbar transpose
TV = 48               # valid out tiles (n0 <= 95)

# ARMA dims (Fin, Fout, K)
LDIMS = [(94, 128, 2), (128, 128, 2), (128, 128, 2),
         (128, 128, 2), (128, 128, 2), (128, 16, 1)]

TAPS2 = [(dy, dx) for dy in range(3) for dx in range(2)]   # conv2 (3,2)
TAPS3 = [(dy, dx) for dx in range(2) for dy in range(3)]   # s = dx*3+dy, delta = dx*49+dy

_NC_CACHE = {}


def _windows(total, step=512):
    out = []
    o = 0
    while o < total:
        out.append((o, min(step, total - o)))
        o += step
    return out


def build_nc(debug=False):
    key = bool(debug)
    if key in _NC_CACHE:
        return _NC_CACHE[key]
    nc = bacc.Bacc("TRN2", target_bir_lowering=False, debug=False)

    # ---- I/O ----
    imgT = nc.dram_tensor("imgT", [IN_PIX, NPAD], F32, kind="ExternalInput")
    m_mat = nc.dram_tensor("m_mat", [NG, NPAD], BF16, kind="ExternalInput")
    deg_pad = nc.dram_tensor("deg_pad", [128, NCH], F32, kind="ExternalInput")
    deg_loc = nc.dram_tensor("deg_loc", [1, NPAD], F32, kind="ExternalInput")
    w1a = nc.dram_tensor("w1a", [9, 100], BF16, kind="ExternalInput")
    b1a = nc.dram_tensor("b1a", [100, 1], F32, kind="ExternalInput")
    w2a = nc.dram_tensor("w2a", [6, 100, 200], BF16, kind="ExternalInput")
    b2a = nc.dram_tensor("b2a", [128, 2], F32, kind="ExternalInput")
    w3a = nc.dram_tensor("w3a", [2, 128, 16], BF16, kind="ExternalInput")
    b3v = nc.dram_tensor("b3v", [1, 1], F32, kind="ExternalInput")
    ident_in = nc.dram_tensor("ident", [128, 128], BF16, kind="ExternalInput")
    awi, awr, ab = [], [], []
    for li, (fi, fo, kk) in enumerate(LDIMS):
        awi.append(nc.dram_tensor(f"wi{li}", [kk, fi, fo], BF16, kind="ExternalInput"))
        awr.append(nc.dram_tensor(f"wr{li}", [kk, fi, fo], BF16, kind="ExternalInput"))
        ab.append(nc.dram_tensor(f"b{li}", [kk, fo], F32, kind="ExternalInput"))
    out_t = nc.dram_tensor("out", [NPAD, 16], F32, kind="ExternalOutput")
    dbg = {}
    if debug:
        dbg["h"] = nc.dram_tensor("dbg_h", [94, NPAD], F32, kind="ExternalOutput")
        dbg["a0"] = nc.dram_tensor("dbg_a0", [128, NPAD], F32, kind="ExternalOutput")
        dbg["z1"] = nc.dram_tensor("dbg_z1", [94, NPAD], F32, kind="ExternalOutput")
        dbg["x2"] = nc.dram_tensor("dbg_x2", [128, NPAD], F32, kind="ExternalOutput")

    with tile.TileContext(nc) as tc:
        with (
            tc.tile_pool(name="pg", bufs=1) as pg,          # persistent SBUF
            tc.tile_pool(name="dram", bufs=1, space="DRAM") as dram,
        ):
            ident = pg.tile([128, 128], BF16, tag="ident")
            nc.sync.dma_start(ident[:], ident_in[:])
            idf = pg.tile([16, 16], F32, tag="idf")
            nc.gpsimd.dma_start(idf[:], ident_in[0:16, 0:16])  # cast to f32
            x1T = pg.tile([94, NPAD], BF16, tag="x1T")  # CNN output h^T

            abuf = dram.tile([NG, NPAD], BF16, tag="abuf")

            # ============ phase A: build A^T in DRAM ============
            with tc.tile_pool(name="pa", bufs=2) as pa:
                dpt = pa.tile([128, NCH], F32, tag="dpt", bufs=1)
                nc.sync.dma_start(dpt[:], deg_pad[:])
                dps = pa.tile([128, NCH], F32, tag="dps", bufs=1)
                nc.scalar.activation(dps[:], dpt[:], AF.Sqrt)
                dinv = pg.tile([128, NCH], F32, tag="dinv")
                nc.vector.reciprocal(dinv[:], dps[:])

                dlt = pa.tile([1, NPAD], F32, tag="dlt", bufs=1)
                nc.sync.dma_start(dlt[:], deg_loc[:])
                dls = pa.tile([1, NPAD], F32, tag="dls", bufs=1)
                nc.scalar.activation(dls[:], dlt[:], AF.Sqrt)
                dlr = pa.tile([1, NPAD], F32, tag="dlr", bufs=1)
                nc.vector.reciprocal(dlr[:], dls[:])
                dinvb = pg.tile([128, NPAD], F32, tag="dinvb")
                nc.gpsimd.partition_broadcast(dinvb[:], dlr[:])

                for c in range(NCH):
                    mt = pa.tile([128, NPAD], BF16, tag="mt")
                    nc.sync.dma_start(mt[:], m_mat[128 * c : 128 * (c + 1), :])
                    at_ = pa.tile([128, NPAD], BF16, tag="at")
                    # (M * dinv_row) * dinv_col
                    nc.vector.scalar_tensor_tensor(
                        at_[:], mt[:], dinv[:, c : c + 1], dinvb[:],
                        op0=ALU.mult, op1=ALU.mult)
                    nc.sync.dma_start(abuf[128 * c : 128 * (c + 1), :], at_[:])
                    if debug and c == 0:
                        nc.gpsimd.dma_start(dbg["a0"][:], at_[:])

            # ============ phase B: CNN ============
            with (
                tc.tile_pool(name="pb", bufs=2) as pb,
                tc.tile_pool(name="pw1", bufs=1) as pw1,
                tc.tile_pool(name="psB", bufs=2, space="PSUM") as psB,
            ):
                w1s = pw1.tile([9, 100], BF16, tag="w1s")
                nc.sync.dma_start(w1s[:], w1a[:])
                b1s = pw1.tile([100, 1], F32, tag="b1s")
                nc.sync.dma_start(b1s[:], b1a[:])
                w2s = pw1.tile([100, 6, 200], BF16, tag="w2s")
                nc.sync.dma_start(
                    w2s[:],
                    bass.AP(tensor=w2a.ap().tensor, offset=0,
                            ap=[[200, 100], [20000, 6], [1, 200]]))
                b2s = pw1.tile([128, 2], F32, tag="b2s")
                nc.sync.dma_start(b2s[:], b2a[:])
                w3s = pw1.tile([128, 2, 16], BF16, tag="w3s")
                nc.sync.dma_start(
                    w3s[:],
                    bass.AP(tensor=w3a.ap().tensor, offset=0,
                            ap=[[16, 128], [2048, 2], [1, 16]]))
                b3bc = pw1.tile([128, 1], F32, tag="b3bc")
                b3t = pw1.tile([1, 1], F32, tag="b3t")
                nc.sync.dma_start(b3t[:], b3v[:])
                nc.gpsimd.partition_broadcast(b3bc[:], b3t[:])

                for b in range(NBLK):
                    # ---- im2col: x0 [9, (x4)(y51)(i64)] ----
                    x0 = pb.tile([9, X1W], BF16, tag="x0")
                    for dy in range(3):
                        for x in range(4):
                            src = bass.AP(
                                tensor=imgT.ap().tensor,
                                offset=(dy * 6 + x) * NPAD + BLK * b,
                                ap=[[NPAD, 3], [6 * NPAD, 51], [1, BLK]])
                            nc.gpsimd.dma_start(
                                x0[3 * dy : 3 * dy + 3,
                                   x * 3264 : (x + 1) * 3264], src)
                    # ---- conv1 ----
                    x1 = pb.tile([100, X1W], BF16, tag="x1")
                    for (o, n) in _windows(X1W):
                        pc1 = psB.tile([100, 512], F32, tag="c1")
                        nc.tensor.matmul(pc1[:, :n], w1s[:], x0[:, o : o + n],
                                         start=True, stop=True)
                        nc.scalar.activation(x1[:, o : o + n], pc1[:, :n],
                                             AF.Lrelu, bias=b1s[:], alpha=0.01)
                    # ---- conv2 ----
                    x2h = [pb.tile([128 if h == 0 else 72, X2W], BF16, tag=f"x2h{h}", name=f"x2h{h}", bufs=1) for h in range(2)]
                    for h in range(2):
                        nc.vector.memset(x2h[h][:, X2V:X2W], 0.0)
                    for (mo, mn) in [(0, 128), (128, 72)]:
                        for x0i in range(3):
                            for (o, n) in _windows(3136):
                                pc2 = psB.tile([128, 512], F32, tag="c2")
                                for ti, (dy, dx) in enumerate(TAPS2):
                                    nc.tensor.matmul(
                                        pc2[:mn, :n],
                                        w2s[:, ti, mo : mo + mn],
                                        x1[:, (x0i + dx) * 3264 + dy * 64 + o :
                                           (x0i + dx) * 3264 + dy * 64 + o + n],
                                        start=(ti == 0), stop=(ti == 5))
                                co = x0i * 3136 + o
                                if mo == 0:
                                    nc.scalar.activation(
                                        x2h[0][0:128, co : co + n],
                                        pc2[0:128, :n], AF.Lrelu,
                                        bias=b2s[0:128, 0:1], alpha=0.01)
                                else:
                                    nc.scalar.activation(
                                        x2h[1][0:72, co : co + n],
                                        pc2[0:72, :n], AF.Lrelu,
                                        bias=b2s[0:72, 1:2], alpha=0.01)
                    # ---- conv3: U psum -> u_sb [16, X2W] ----
                    u_sb = pb.tile([16, X2W], BF16, tag="usb", bufs=1)
                    for (o, n) in _windows(X2W):
                        pc3 = psB.tile([16, 512], F32, tag="c3")
                        nc.tensor.matmul(pc3[:, :n], w3s[0:128, 0, :],
                                         x2h[0][:, o : o + n], start=True, stop=False)
                        nc.tensor.matmul(pc3[:, :n], w3s[0:72, 1, :],
                                         x2h[1][0:72, o : o + n], start=False, stop=True)
                        nc.vector.tensor_copy(u_sb[:, o : o + n], pc3[:, :n])
                    # ---- xbar transpose: utt [128, 74, 16] ----
                    utt = pb.tile([128, NT2, 16], BF16, tag="utt")
                    nc.sync.dma_start(utt[:], u_sb[:], transpose=True)

                    # ---- tap-sum tree over shifts ----
                    def view(delta, hd, s):
                        hs = (hd + delta) % 2
                        toff = (hd + delta) // 2
                        return utt[64 * hs : 64 * hs + 64, toff : toff + TV, s]

                    # walrus: both SB inputs of a 2-input op must share the
                    # base partition, so group taps by shift parity.
                    # evens: d=0 (s0), d=2 (s2), d=50 (s4) at base 64*hd
                    # odds:  d=1 (s1), d=49 (s3), d=51 (s5) at base 64*(1-hd)
                    tA = pb.tile([128, TV], F32, tag="tA")
                    tB = pb.tile([128, TV], F32, tag="tB")
                    tC = pb.tile([128, TV], F32, tag="tC")
                    tD = pb.tile([128, TV], F32, tag="tD")
                    tE = pb.tile([128, TV], F32, tag="tE")
                    for hd in range(2):
                        p0 = slice(64 * hd, 64 * hd + 64)
                        q0 = slice(64 * (1 - hd), 64 * (1 - hd) + 64)
                        nc.vector.tensor_tensor(tA[p0, :], view(0, hd, 0),
                                                view(2, hd, 2), ALU.add)
                        nc.vector.tensor_tensor(tA[p0, :], tA[p0, :],
                                                view(50, hd, 4), ALU.add)
                        nc.vector.tensor_tensor(tB[q0, :], view(1, hd, 1),
                                                view(49, hd, 3), ALU.add)
                        nc.vector.tensor_tensor(tB[q0, :], tB[q0, :],
                                                view(51, hd, 5), ALU.add)
                        nc.vector.tensor_copy(tD[p0, :], tB[q0, :])
                        nc.vector.tensor_tensor(tE[p0, :], tA[p0, :],
                                                tD[p0, :], ALU.add)

                    # ---- extract valid + tanh(+b3) -> hblk [64, 94] ----
                    hblk = pb.tile([64, 94], BF16, tag="hblk")
                    for (hd, t0, cnt, f0) in [(0, 0, 24, 0), (1, 0, 23, 2),
                                              (1, 24, 24, 1), (0, 25, 23, 3)]:
                        srcv = tE[64 * hd : 64 * hd + 64, t0 : t0 + cnt]
                        dstv = bass.AP(tensor=hblk.tensor,
                                       offset=hblk.offset + f0,
                                       ap=[list(hblk.ap[0]), [4, cnt]])
                        nc.scalar.activation(
                            dstv, srcv, AF.Tanh,
                            bias=b3bc[64 * hd : 64 * hd + 64, :])
                    # ---- transpose -> x1T[:, 64b:64b+64] ----
                    hp = psB.tile([94, 64], BF16, tag="ht")
                    nc.tensor.transpose(hp[:], hblk[:], ident[0:64, 0:64])
                    nc.vector.tensor_copy(x1T[:, BLK * b : BLK * (b + 1)], hp[:])

            if debug:
                nc.gpsimd.dma_start(dbg["h"][:], x1T[:])

            # ============ phase C: GNN ============
            with (
                tc.tile_pool(name="pgA", bufs=1) as pgA,
                tc.tile_pool(name="pwg", bufs=3) as pwg,
                tc.tile_pool(name="psG", bufs=1, space="PSUM") as psG,
            ):
                at = pgA.tile([128, NCH, NPAD], BF16, tag="abig")
                nc.sync.dma_start(
                    at[:],
                    bass.AP(tensor=abuf.tensor, offset=abuf.offset,
                            ap=[[NPAD, 128], [128 * NPAD, NCH], [1, NPAD]]))
                xT = x1T
                for li, (fi, fo, kk) in enumerate(LDIMS):
                    # weights
                    wis = pwg.tile([fi, kk, fo], BF16, tag=f"wi{li}", bufs=1)
                    nc.sync.dma_start(
                        wis[:],
                        bass.AP(tensor=awi[li].ap().tensor, offset=0,
                                ap=[[fo, fi], [fi * fo, kk], [1, fo]]))
                    wrs = pwg.tile([fi, kk, fo], BF16, tag=f"wr{li}", bufs=1)
                    nc.sync.dma_start(
                        wrs[:],
                        bass.AP(tensor=awr[li].ap().tensor, offset=0,
                                ap=[[fo, fi], [fi * fo, kk], [1, fo]]))
                    bs = pwg.tile([fo, kk], F32, tag=f"bs{li}", bufs=1)
                    for k in range(kk):
                        nc.sync.dma_start(
                            bs[:, k : k + 1],
                            bass.AP(tensor=ab[li].ap().tensor, offset=k * fo,
                                    ap=[[1, fo], [1, 1]]))

                    # local node-major chunks -> AG input
                    agin = dram.tile([NPAD, fi], BF16, tag=f"agin{li}")
                    agout = dram.tile([NG, fi], BF16, tag=f"agout{li}", addr_space="Shared")
                    for c in range(NPAD // 128):
                        tp = psG.tile([128, fi], BF16, tag="gt", bufs=2)
                        nc.tensor.transpose(
                            tp[:], xT[0:fi, 128 * c : 128 * (c + 1)],
                            ident[0:fi, 0:fi])
                        xs = pwg.tile([128, fi], BF16, tag="xs")
                        nc.vector.tensor_copy(xs[:], tp[:])
                        nc.sync.dma_start(agin[128 * c : 128 * (c + 1), :], xs[:])
                    nc.gpsimd.collective_compute(
                        "AllGather", ALU.bypass,
                        ins=[agin.opt()], outs=[agout.opt()],
                        replica_groups=[list(range(N_CORES))])

                    # aggregate: z^T [fi, 640] = sum_c x_chunk(c).T @ A^T(c)
                    zp = psG.tile([fi, NPAD], F32, tag="zp", bufs=1)
                    for c in range(NCH):
                        xc = pwg.tile([128, fi], BF16, tag="xc")
                        nc.sync.dma_start(xc[:], agout[128 * c : 128 * (c + 1), :])
                        for (o, n) in _windows(NPAD):
                            nc.tensor.matmul(
                                zp[:, o : o + n], xc[:],
                                at[:, c, o : o + n],
                                start=(c == 0), stop=(c == NCH - 1))
                    zt = pwg.tile([fi, NPAD], BF16, tag="zt")
                    nc.vector.tensor_copy(zt[:], zp[:])
                    if debug and li == 0:
                        nc.gpsimd.dma_start(dbg["z1"][:], zt[:])

                    # ARMA stacks
                    rks = []
                    for k in range(kk):
                        pr = psG.tile([fo, NPAD], F32, tag="pr", bufs=1)
                        for (o, n) in _windows(NPAD):
                            nc.tensor.matmul(pr[:, o : o + n], wis[:, k, :],
                                             zt[:, o : o + n], start=True, stop=False)
                            nc.tensor.matmul(pr[:, o : o + n], wrs[:, k, :],
                                             xT[0:fi, o : o + n], start=False, stop=True)
                        rk = pwg.tile([fo, NPAD], F32, tag=f"rk{k}")
                        nc.scalar.activation(rk[:], pr[:], AF.Relu,
                                             bias=bs[:, k : k + 1])
                        rks.append(rk)
                    if li < 5:
                        ssum = pwg.tile([fo, NPAD], F32, tag="ssum")
                        nc.vector.tensor_tensor(ssum[:], rks[0][:], rks[1][:], ALU.add)
                        newx = pg.tile([fo, NPAD], BF16, tag=f"xT{li}")
                        nc.scalar.activation(newx[:], ssum[:], AF.Tanh, scale=0.5)
                        xT = newx
                        if debug and li == 0:
                            nc.gpsimd.dma_start(dbg["x2"][:], newx[:])
                    else:
                        out6 = rks[0]  # [16, 640] f32

                # ---- softmax over the 16 features ----
                for c in range(NPAD // 128):
                    sp = psG.tile([128, 16], F32, tag="sm", bufs=2)
                    nc.tensor.transpose(sp[:], out6[:, 128 * c : 128 * (c + 1)],
                                        idf[:])
                    sb = pwg.tile([128, 16], F32, tag="sb")
                    nc.vector.tensor_copy(sb[:], sp[:])
                    mx = pwg.tile([128, 1], F32, tag="mx")
                    nc.vector.tensor_reduce(mx[:], sb[:], axis=mybir.AxisListType.X,
                                            op=ALU.max)
                    nmx = pwg.tile([128, 1], F32, tag="nmx")
                    nc.vector.tensor_scalar_mul(nmx[:], mx[:], -1.0)
                    ex = pwg.tile([128, 16], F32, tag="ex")
                    nc.scalar.activation(ex[:], sb[:], AF.Exp, bias=nmx[:])
                    sm = pwg.tile([128, 1], F32, tag="smr")
                    nc.vector.tensor_reduce(sm[:], ex[:], axis=mybir.AxisListType.X,
                                            op=ALU.add)
                    rc = pwg.tile([128, 1], F32, tag="rc")
                    nc.vector.reciprocal(rc[:], sm[:])
                    so = pwg.tile([128, 16], F32, tag="so")
                    nc.vector.tensor_scalar_mul(so[:], ex[:], rc[:])
                    nc.sync.dma_start(out_t[128 * c : 128 * (c + 1), :], so[:])

    nc.compile()
    _NC_CACHE[key] = nc
    return nc


# ================= host side =================

def host_prep(inputs):
    """Build the 8 per-core input maps from the full problem inputs."""
    bf = ml_dtypes.bfloat16
    imgs = np.asarray(inputs["inputs"], np.float32).reshape(N_NODES, IN_PIX)
    ei = np.asarray(inputs["edge_index"])
    src, dst = ei[0].astype(np.int64), ei[1].astype(np.int64)

    deg = np.bincount(dst, minlength=N_NODES).astype(np.int64)
    keep = (deg[src] > 0) & (deg[dst] > 0)
    srcK, dstK = src[keep], dst[keep]
    spad = (srcK // NLOC) * NPAD + (srcK % NLOC)

    degc = np.maximum(deg, 1).astype(np.float32)
    sprime = np.arange(NG)
    off = sprime % NPAD
    real = (sprime // NPAD) * NLOC + np.minimum(off, NLOC - 1)
    dp_flat = np.ones(NG, np.float32)
    ok = off < NLOC
    dp_flat[ok] = degc[real[ok]]
    deg_pad_np = dp_flat.reshape(NCH, 128).T.copy()  # s' = c*128+p

    w1a = np.ascontiguousarray(
        np.asarray(inputs["e1_w"], np.float32)[:, 0, :, :]
        .transpose(1, 2, 0).reshape(9, 100))
    b1a = np.asarray(inputs["e1_b"], np.float32).reshape(100, 1)
    e2w = np.asarray(inputs["e2_w"], np.float32)  # [200, 100, 3, 2]
    w2a = np.zeros((6, 100, 200), np.float32)
    for t, (dy, dx) in enumerate(TAPS2):
        w2a[t] = e2w[:, :, dy, dx].T
    b2r = np.asarray(inputs["e2_b"], np.float32)
    b2a = np.zeros((128, 2), np.float32)
    b2a[:, 0] = b2r[0:128]
    b2a[0:72, 1] = b2r[128:200]
    e3w = np.asarray(inputs["e3_w"], np.float32)  # [1, 200, 3, 2]
    w3a = np.zeros((2, 128, 16), np.float32)
    for si, (dy, dx) in enumerate(TAPS3):
        w3a[0, :, si] = e3w[0, 0:128, dy, dx]
        w3a[1, 0:72, si] = e3w[0, 128:200, dy, dx]
    b3v = np.asarray(inputs["e3_b"], np.float32).reshape(1, 1)
    ident = np.eye(128, dtype=np.float32)

    shared = {
        "deg_pad": deg_pad_np,
        "w1a": w1a.astype(bf), "b1a": b1a,
        "w2a": w2a.astype(bf), "b2a": b2a,
        "w3a": w3a.astype(bf), "b3v": b3v,
        "ident": ident.astype(bf),
    }
    for li in range(6):
        shared[f"wi{li}"] = np.asarray(inputs[f"a{li+1}_wi"], np.float32).astype(bf)
        shared[f"wr{li}"] = np.asarray(inputs[f"a{li+1}_wr"], np.float32).astype(bf)
        shared[f"b{li}"] = np.asarray(inputs[f"a{li+1}_b"], np.float32)

    in_maps = []
    for k in range(N_CORES):
        lo, hi = k * NLOC, (k + 1) * NLOC
        it = np.zeros((IN_PIX, NPAD), np.float32)
        it[:, :NLOC] = imgs[lo:hi].T
        mk = np.zeros((NG, NPAD), np.float32)
        sel = (dstK >= lo) & (dstK < hi)
        np.add.at(mk, (spad[sel], dstK[sel] - lo), 1.0)
        dl = np.ones((1, NPAD), np.float32)
        dl[0, :NLOC] = degc[lo:hi]
        m = dict(shared)
        m["imgT"] = it
        m["m_mat"] = mk.astype(bf)
        m["deg_loc"] = dl
        in_maps.append(m)
    return in_maps


def kernel(**inputs):
    nc = build_nc(debug=False)
    in_maps = host_prep(inputs)
    res = bass_utils.run_bass_kernel_spmd(
        nc, in_maps, core_ids=list(range(N_CORES)))
    out = np.concatenate(
        [res.results[k]["out"][:NLOC] for k in range(N_CORES)], axis=0)
    return out.astype(np.float32)
